# revision 26
# baseline (speedup 1.0000x reference)
"""Biaffine kernel for Trainium2, data-parallel over batch across 8 NeuronCores.

Reference math (per batch b):
    Daug = [D, 1]                                  # [S, d+1]
    out  = Daug @ U @ H^T + (Daug @ W[:d+1])[:, None] + (H @ W[d+1:])[None, :]

Algebraic refactor used here (d = 1024):
    U0 = U[:d]                # [d, d]
    c  = U[d] + W[d+1:]       # [d]  (folds the ones-row of Daug and the H linear term)
    T' = D @ U0 + c           # [S, d]
    dlin = D @ W[:d] + W[d]   # [S]  (tiny; computed host-side)
    out  = T' @ H^T + dlin[:, None]

Device kernel per core (4 batches, 384 matmuls):
    matmul1: T'^T[j, x] = sum_k U0[k, j] * D^T[k, x]  (lhsT = U0, rhs = D^T)
             + per-partition bias c fused into the PSUM->SBUF copy (DVE)
    matmul2: out[x, y] = sum_j T'^T[j, x] * H^T[j, y] (lhsT = T'^T, rhs = H^T)
             + per-partition bias dlin fused into the PSUM->SBUF copy (DVE)

Performance structure (measured on trn2):
  - Matmul operands are float16 (fp32 PSUM accumulation). Measured PE issue
    rates at N=512: fp16 ~216 ns/matmul (streaming floor 213), fp32r ~227,
    bf16 ~259 — fp16 is both the fastest dtype on this silicon AND half the
    DMA bytes of fp32r. PE floor: 384 matmuls x 512 rows @ 2.4 GHz = 81.9 us.
    End-to-end relative error ~4.6e-4 (fp16 input/T' quantization, validated
    against a host emulation). The output is stored as fp16 (adds ~2.4e-4)
    and upconverted host-side, halving store traffic.
  - D^T / H^T / U0 are transposed AND pre-swizzled host-side to the exact SBUF
    layout, so the device does zero transposes and every DMA partition read is
    one contiguous block.
  - Startup is device-HBM-contention bound (all 8 cores pull their batch-0
    data at once; per-ring delivery ~25 GB/s in the first 5 us, ~110-130
    after). Batch 0 loads in strict consumption order, alternating rings per
    kt chunk; the first u0 chunk is halved so the first data matmul starts
    ~1 us earlier. Batches 1-3 load as 1-2 DMAs per tensor (each DMA
    instruction costs ~0.6 us of queue-issue time).
  - Batch 0's matmul1 runs kt-outer over jm 0-5 (6 PSUM banks) with 2 filler
    matmuls per kt step (the phase is DMA-paced; fillers keep the HAM clock
    gate at full speed), then jm 6-7 as whole chains: their stops stagger so
    the single DVE (~740 ns per PSUM->SBUF copy) drains banks 0-5 while the
    PE still has useful work, and matmul2 never waits on a copy. ~10 warm-up
    matmuls on a memset tile bridge the preamble->first-data window (~7.8 to
    ~12 us) so the clock is warm when real matmuls begin.
  - Stores alternate rings (xt even: scalar, xt odd: sync); the final output
    tile is computed as a half + two quarter chains in separate PSUM banks so
    the drain tail ends on a 32 KB store.

BIAFFINE_MM=f32r switches to fp32r matmuls; BIAFFINE_MM=f32 switches to
exact fp32 matmuls (~3x slower, rel err ~5e-7). BIAFFINE_U0_BF16=1 loads U0
as bf16 (mixed 16-bit operand dtypes work on the PE; same bytes, more error
— off by default).
"""
import os
import sys

import numpy as np

for _p in (
    "/root/.axon_site",
    "/root/.axon_site/_ro/trn_rl_repo",
    "/root/.axon_site/_ro/pypackages",
    "/opt/trn_rl_repo",
):
    if os.path.isdir(_p) and _p not in sys.path:
        sys.path.append(_p)

import concourse.bass as bass
import concourse.mybir as mybir
import concourse.tile as tile
from concourse import bacc
from concourse.bass_utils import run_bass_kernel_spmd

B, S, D_DIM = 32, 512, 1024
N_CORES = 8
BPC = B // N_CORES  # batches per core
KT = D_DIM // 128  # 8 k-tiles (contraction over d)
JT = D_DIM // 128  # 8 j-tiles (M dim of matmul1)
XT = S // 128  # 4 x-tiles (M dim of matmul2)

_NC_CACHE = {}


def _mode() -> str:
    m = os.environ.get("BIAFFINE_MM", "f16")
    assert m in ("f16", "bf16", "f32r", "f32"), m
    return m


def _build_nc(mode: str) -> bass.Bass:
    nc = bacc.Bacc()
    f32 = mybir.dt.float32
    mm_dt = {
        "f16": mybir.dt.float16,
        "bf16": mybir.dt.bfloat16,
        "f32r": mybir.dt.float32r,
        "f32": f32,
    }[mode]
    n_warm = int(os.environ.get("BIAFFINE_WARM", "10"))
    # Store the output in the matmul dtype (fp16/bf16): halves store traffic
    # and the drain-tail store; the host upconverts to fp32. Output range
    # (|out| < ~200) is far inside fp16 range; adds ~2.4e-4 quantization.
    out_dt = mm_dt if mode in ("f16", "bf16") else f32
    # Optional: u0 in bf16 (same 2 bytes — no DMA saving; kept only as a
    # demonstration that the PE accepts mixed 16-bit weight/moving dtypes).
    # Adds bf16 quantization on U0 (~1.6e-3 end-to-end); off by default.
    u0_dt = (
        mybir.dt.bfloat16
        if mode == "f16" and os.environ.get("BIAFFINE_U0_BF16", "0") == "1"
        else mm_dt
    )

    # Inputs arrive pre-swizzled to the SBUF layout: [.., p, kt, x] so each
    # partition's DMA read is one contiguous block.
    dt_in = nc.dram_tensor("dt_in", [BPC, 128, KT, S], mm_dt, kind="ExternalInput")
    ht_in = nc.dram_tensor("ht_in", [BPC, 128, KT, S], mm_dt, kind="ExternalInput")
    u0_in = nc.dram_tensor("u0_in", [128, KT, D_DIM], u0_dt, kind="ExternalInput")
    ccol_in = nc.dram_tensor("ccol_in", [128, JT], f32, kind="ExternalInput")
    dcol_in = nc.dram_tensor("dcol_in", [128, BPC * XT], f32, kind="ExternalInput")
    out_t = nc.dram_tensor("out", [BPC, S, S], out_dt, kind="ExternalOutput")

    with tile.TileContext(nc) as tc:
        with (
            tc.tile_pool(name="const", bufs=1) as cpool,
            tc.tile_pool(name="dh", bufs=2) as dh_pool,
            tc.tile_pool(name="tt", bufs=2) as tt_pool,
            tc.tile_pool(name="ot", bufs=3) as ot_pool,
            tc.tile_pool(name="ps", bufs=8, space="PSUM") as ps_pool,
        ):
            # HAM warm-up: a few matmuls on a memset tile fill the startup DMA
            # window with real array work so the PE is at the warm clock when
            # the first data matmul issues. (gpsimd memset: it's idle early.)
            warm_sb = cpool.tile([128, S], mm_dt, name="warm_sb")
            nc.gpsimd.memset(warm_sb[:], 0.0)
            warm_ps = ps_pool.tile([128, S], f32, tag="ps", bufs=7, name="warm_ps")
            fill_ps = ps_pool.tile([128, S], f32, tag="fill", bufs=1, name="fill_ps")
            for _ in range(n_warm):
                nc.tensor.matmul(
                    warm_ps[:], lhsT=warm_sb[:, :128], rhs=warm_sb[:], start=True,
                    stop=True,
                )

            # Merged SBUF tiles (u0 flat [128, KT*D_DIM]; batch-0 D^T/H^T
            # use the same full-batch layout as batches 1-3), loaded by
            # chunked DMAs in strict consumption order, alternating rings by
            # kt parity so both rings carry ~equal bytes per kt step. Each
            # DMA instruction costs ~0.6 us of queue-issue time, so chunks
            # are as coarse as the consumption deadlines allow: per-kt for
            # u0/dt0 (matmul1 is paced by them), halves for ht0 (needed all
            # at once at matmul2). The tiny bias columns ride along after
            # the first chunk (the first tt copy needs ccol by ~22 us).
            u0_t = cpool.tile([128, KT * D_DIM], u0_dt, name="u0_t")
            ccol = cpool.tile([128, JT], f32)
            dcol = cpool.tile([128, BPC * XT], f32)
            dt0 = dh_pool.tile([128, KT * S], mm_dt, tag="dtf", name="dtf0")
            ht0 = dh_pool.tile([128, KT * S], mm_dt, tag="htf", name="htf0")
            u0s = u0_in.rearrange("p k d -> p (k d)")
            dts0 = dt_in[0].rearrange("p k x -> p (k x)")
            hts0 = ht_in[0].rearrange("p k x -> p (k x)")
            # Strict consumption-order interleave, alternating rings by kt
            # parity so both rings carry ~equal bytes per kt step (batch 0's
            # matmul1 consumes one (u0[k], dt0[k]) pair per ~1.7 us step).
            # The first u0 chunk is halved for the earliest possible start;
            # the tiny bias columns ride along right after the first chunks
            # (the first tt copy needs ccol by ~22 us). ht0 goes last as one
            # half per ring (needed all at once at matmul2).
            hd = D_DIM // 2
            nc.sync.dma_start(u0_t[:, :hd], u0s[:, :hd])
            nc.scalar.dma_start(dt0[:, :S], dts0[:, :S])
            nc.sync.dma_start(u0_t[:, hd:D_DIM], u0s[:, hd:D_DIM])
            nc.sync.dma_start(ccol[:], ccol_in[:])
            nc.scalar.dma_start(u0_t[:, D_DIM : 2 * D_DIM], u0s[:, D_DIM : 2 * D_DIM])
            nc.scalar.dma_start(dcol[:], dcol_in[:])
            nc.sync.dma_start(dt0[:, S : 2 * S], dts0[:, S : 2 * S])
            for kt in range(2, KT):
                a, b_ = (nc.sync, nc.scalar) if kt % 2 == 0 else (nc.scalar, nc.sync)
                a.dma_start(
                    u0_t[:, kt * D_DIM : (kt + 1) * D_DIM],
                    u0s[:, kt * D_DIM : (kt + 1) * D_DIM],
                )
                b_.dma_start(dt0[:, kt * S : (kt + 1) * S], dts0[:, kt * S : (kt + 1) * S])
            hw0 = KT * S // 2
            nc.scalar.dma_start(ht0[:, :hw0], hts0[:, :hw0])
            nc.sync.dma_start(ht0[:, hw0:], hts0[:, hw0:])

            def u0j(kt, jm):
                return u0_t[:, kt * D_DIM + jm * 128 : kt * D_DIM + (jm + 1) * 128]

            dt_full, ht_full = dt0, ht0
            for b in range(BPC):
                # Prefetch batch b+1 as one DMA per tensor (dt: sync ring,
                # ht: scalar ring); emitted before this batch's stores so the
                # loads aren't queued behind store-data-ready waits.
                nxt_dt, nxt_ht = None, None
                if b + 1 < BPC:
                    nxt_dt = dh_pool.tile([128, KT * S], mm_dt, tag="dtf", name="dtf")
                    nxt_ht = dh_pool.tile([128, KT * S], mm_dt, tag="htf", name="htf")
                    dsrc = dt_in[b + 1].rearrange("p k x -> p (k x)")
                    hsrc = ht_in[b + 1].rearrange("p k x -> p (k x)")
                    hw = KT * S // 2
                    if b == 0:
                        # Batch 1 is consumed right on the heels of the
                        # startup loads: split it so the first half (kt 0-3)
                        # lands before batch-1 matmul1 reaches it (subtile
                        # deps let those matmuls start on the half).
                        nc.sync.dma_start(nxt_dt[:, :hw], dsrc[:, :hw])
                        nc.sync.dma_start(nxt_dt[:, hw:], dsrc[:, hw:])
                        nc.scalar.dma_start(nxt_ht[:, :hw], hsrc[:, :hw])
                        nc.scalar.dma_start(nxt_ht[:, hw:], hsrc[:, hw:])
                    else:
                        nc.sync.dma_start(nxt_dt[:], dsrc)
                        nc.scalar.dma_start(nxt_ht[:], hsrc)

                dt_rhs = [dt_full[:, kt * S : (kt + 1) * S] for kt in range(KT)]
                ht_rhs = [ht_full[:, kt * S : (kt + 1) * S] for kt in range(KT)]

                # matmul1: T'^T[jm*128+p, x]  (+ bias c)
                tt_t = [
                    tt_pool.tile([128, S], mm_dt, tag=f"tt{jm}", name=f"tt{jm}")
                    for jm in range(JT)
                ]
                if b == 0:
                    # Head: kt-outer over jm 0-5 (6 live PSUM banks), each kt
                    # step needing only chunk kt of u0/dt so the PE tracks
                    # DMA arrivals. jm 6-7 run as whole chains at the end:
                    # their stops stagger so the DVE (one ~740 ns copy per
                    # bank) drains banks 0-5 while the PE still has useful
                    # work, and matmul2 never waits on a copy.
                    head = 6
                    ps_l = [
                        ps_pool.tile([128, S], f32, tag="ps", bufs=7, name=f"ps{jm}")
                        for jm in range(head)
                    ]
                    for jm in range(head):
                        nc.tensor.matmul(
                            ps_l[jm][:],
                            lhsT=u0j(0, jm),
                            rhs=dt_rhs[0],
                            start=True,
                            stop=False,
                        )
                    for kt in range(1, KT):
                        # Batch 0's head phase is DMA-paced (~1.7 us/step vs
                        # ~1.3 us of matmuls): two fillers per step keep the
                        # PE busy so the HAM clock gate stays at full speed
                        # (a single idle window re-throttles to the mid
                        # clock, which would make the whole phase PE-bound).
                        for _ in range(2):
                            nc.tensor.matmul(
                                fill_ps[:], lhsT=warm_sb[:, :128], rhs=warm_sb[:],
                                start=True, stop=True,
                            )
                        for jm in range(head):
                            nc.tensor.matmul(
                                ps_l[jm][:],
                                lhsT=u0j(kt, jm),
                                rhs=dt_rhs[kt],
                                start=False,
                                stop=(kt == KT - 1),
                            )
                    for jm in range(head):
                        nc.vector.tensor_scalar_add(
                            tt_t[jm][:], ps_l[jm][:], ccol[:, jm : jm + 1]
                        )
                    for jm in range(head, JT):
                        ps = ps_pool.tile([128, S], f32, tag="ps", bufs=7, name="ps")
                        for kt in range(KT):
                            nc.tensor.matmul(
                                ps[:],
                                lhsT=u0j(kt, jm),
                                rhs=dt_rhs[kt],
                                start=(kt == 0),
                                stop=(kt == KT - 1),
                            )
                        nc.vector.tensor_scalar_add(
                            tt_t[jm][:], ps[:], ccol[:, jm : jm + 1]
                        )
                else:
                    for jm in range(JT):
                        ps = ps_pool.tile([128, S], f32, tag="ps", bufs=7, name="ps")
                        for kt in range(KT):
                            nc.tensor.matmul(
                                ps[:],
                                lhsT=u0j(kt, jm),
                                rhs=dt_rhs[kt],
                                start=(kt == 0),
                                stop=(kt == KT - 1),
                            )
                        nc.vector.tensor_scalar_add(
                            tt_t[jm][:], ps[:], ccol[:, jm : jm + 1]
                        )

                # matmul2: out[xt*128+p, y]  (+ bias dlin)
                for xt in range(XT):
                    last_tile = b == BPC - 1 and xt == XT - 1
                    if not last_tile:
                        po = ps_pool.tile([128, S], f32, tag="ps", bufs=7, name="po")
                        for jm in range(JT):
                            nc.tensor.matmul(
                                po[:],
                                lhsT=tt_t[jm][:, xt * 128 : (xt + 1) * 128],
                                rhs=ht_rhs[jm],
                                start=(jm == 0),
                                stop=(jm == JT - 1),
                            )
                        ot = ot_pool.tile([128, S], out_dt, tag="ot", name="ot")
                        nc.vector.tensor_scalar_add(
                            ot[:], po[:], dcol[:, b * XT + xt : b * XT + xt + 1]
                        )
                        # Alternate store rings to balance bytes.
                        eng = nc.scalar if xt % 2 == 0 else nc.sync
                        eng.dma_start(out_t[b, xt * 128 : (xt + 1) * 128, :], ot[:])
                    else:
                        # Final tile: a half-chain then two quarter
                        # chains, each in its own PSUM bank, DVE+store per
                        # piece overlapping the next piece's matmuls — the
                        # drain tail ends on a 32 KB store instead of 128 KB.
                        for lo, hi, eng in (
                            (0, 256, nc.scalar),
                            (256, 384, nc.sync),
                            (384, 512, None),
                        ):
                            po = ps_pool.tile([128, S], f32, tag="ps", bufs=7, name="po")
                            for jm in range(JT):
                                nc.tensor.matmul(
                                    po[:, lo:hi],
                                    lhsT=tt_t[jm][:, xt * 128 : (xt + 1) * 128],
                                    rhs=ht_rhs[jm][:, lo:hi],
                                    start=(jm == 0),
                                    stop=(jm == JT - 1),
                                )
                            ot = ot_pool.tile([128, hi - lo], out_dt, tag="oth", name="oth")
                            nc.vector.tensor_scalar_add(
                                ot[:], po[:, lo:hi], dcol[:, b * XT + xt : b * XT + xt + 1]
                            )
                            if eng is not None:
                                eng.dma_start(
                                    out_t[b, xt * 128 : (xt + 1) * 128, lo:hi], ot[:]
                                )
                            else:
                                # Very last piece: split across both rings so
                                # the two 16 KB stores issue in parallel.
                                mid = (hi - lo) // 2
                                nc.scalar.dma_start(
                                    out_t[b, xt * 128 : (xt + 1) * 128, lo : lo + mid],
                                    ot[:, :mid],
                                )
                                nc.sync.dma_start(
                                    out_t[b, xt * 128 : (xt + 1) * 128, lo + mid : hi],
                                    ot[:, mid:],
                                )

                if nxt_dt is not None:
                    dt_full, ht_full = nxt_dt, nxt_ht
    nc.finalize()
    return nc


def _get_nc() -> bass.Bass:
    key = f"nc_{_mode()}"
    if key not in _NC_CACHE:
        _NC_CACHE[key] = _build_nc(_mode())
    return _NC_CACHE[key]


def _round_fp32r(a: np.ndarray) -> np.ndarray:
    """Round fp32 to fp32r layout: RNE to 11-bit mantissa, low 12 bits zero."""
    bits = np.ascontiguousarray(a, dtype=np.float32).view(np.uint32)
    odd = (bits >> 12) & np.uint32(1)
    out = (bits + np.uint32(0x7FF) + odd) & np.uint32(0xFFFFF000)
    return out.view(np.float32)


def kernel(D, H, U, W):
    D = np.ascontiguousarray(np.asarray(D, dtype=np.float32))
    H = np.ascontiguousarray(np.asarray(H, dtype=np.float32))
    U = np.asarray(U, dtype=np.float32)
    W = np.asarray(W, dtype=np.float32)
    d = D_DIM
    mode = _mode()
    np_mm = np.dtype(
        mybir.dt.np(
            {
                "f16": mybir.dt.float16,
                "bf16": mybir.dt.bfloat16,
                "f32r": mybir.dt.float32r,
                "f32": mybir.dt.float32,
            }[mode]
        )
    )

    def to_mm(a: np.ndarray) -> np.ndarray:
        if mode == "f32r":
            return _round_fp32r(a)
        return np.ascontiguousarray(a).astype(np_mm)

    # U0 swizzled to [128, KT, d]: [p, kt, j] = U0[kt*128+p, j]
    U0 = np.ascontiguousarray(U[:d, :].reshape(KT, 128, d).transpose(1, 0, 2))
    if mode == "f16" and os.environ.get("BIAFFINE_U0_BF16", "0") == "1":
        U0 = U0.astype(np.dtype(mybir.dt.np(mybir.dt.bfloat16)))
    else:
        U0 = to_mm(U0)
    c = (U[d, :] + W[d + 1 :]).astype(np.float32)  # [d]
    # ccol[p, jm] = c[jm*128 + p]
    ccol = np.ascontiguousarray(c.reshape(JT, 128).T)
    # dlin[b, x] = D[b, x] . W[:d] + W[d]  (from unrounded fp32 D: exact)
    dlin = (D @ W[:d] + W[d]).astype(np.float32)  # [B, S]

    in_maps = []
    for cidx in range(N_CORES):
        sl = slice(cidx * BPC, (cidx + 1) * BPC)
        # [b, p, kt, x] = X[b, x, kt*128+p]  (transpose + swizzle in one copy)
        Dt = to_mm(D[sl].reshape(BPC, S, KT, 128).transpose(0, 3, 2, 1))
        Ht = to_mm(H[sl].reshape(BPC, S, KT, 128).transpose(0, 3, 2, 1))
        # dcol[p, b*XT + xt] = dlin[b, xt*128 + p]
        dcol = np.ascontiguousarray(
            dlin[sl].reshape(BPC, XT, 128).transpose(2, 0, 1).reshape(128, BPC * XT)
        )
        in_maps.append(
            {
                "dt_in": Dt,
                "ht_in": Ht,
                "u0_in": U0,
                "ccol_in": ccol,
                "dcol_in": dcol,
            }
        )

    nc = _get_nc()
    trace = bool(int(os.environ.get("BIAFFINE_TRACE", "0")))
    kwargs = {}
    if trace:
        tdir = os.environ.get("BIAFFINE_TRACE_DIR")
        if tdir:
            os.makedirs(tdir, exist_ok=True)
            kwargs["tmpdir"] = tdir
    res = run_bass_kernel_spmd(
        nc, in_maps, core_ids=list(range(N_CORES)), trace=trace, **kwargs
    )
    if trace and res.exec_time_ns is not None:
        print(f"HW exec time: {res.exec_time_ns} ns")

    out = np.concatenate([res.results[i]["out"] for i in range(N_CORES)], axis=0)
    return np.ascontiguousarray(out.astype(np.float32))
